# revision 9
# baseline (speedup 1.0000x reference)
"""Multi-head attention (B=2, S=2048, E=1024, H=16, D=64) on 8 TRN2 cores.

Sharding: core c = b*4 + g  ->  batch b in {0,1}, head-group g in {0..3}
(4 heads = 256 embed columns per group).  Each core computes its group's
Q/K/V projections, attention, and the partial output projection
(outT [1024, 2048], the Wo[:, group]-contracted context).  Host sums the
4 group partials per batch, transposes, and adds bo.

Key structure (v2):
- All DRAM params partition-folded host-side so each loads in one big DMA.
- Q^T/K^T stored head-dim-major [128, 2048] per head-pair t; the scores
  matmul contracts K=64 per head using PE row tiling (tile_position rows
  0-63 / 64-127), so the two heads of a pair run CONCURRENTLY in the
  128x128 array -- no zero-padding waste.
- Score PSUM is a global stream of [128, 512] banks (one per (kt, par)),
  chunked into 3-bank tiles double-buffered in 6 PSUM banks.  exp runs as
  one ACT call per same-reader bank run (bigger calls amortize the
  352-cycle ACT overhead).
- Optionally some heads' exp is offloaded to the vector engine as a
  Schraudolph bf16 exp (single tensor_scalar: i16 = round(s*A + B),
  bit-interpreted as bf16), relieving the ACT throughput floor.
- attn.V: lhsT = exp^T chunk [128k, 128q], rhs = [V|1] [128k, 65]; PSUM
  col 64 = softmax denominator; normalize = reciprocal + tensor_scalar_mul.
- context is PE-transposed in 128x128 blocks for the output projection.
- A software scheduler interleaves projection / attn.V / transpose /
  out-proj work between score units to keep PE busy at the ACT call pace.
"""

import sys

import numpy as np

_REPO = "/opt/trn_rl_repo"
if _REPO not in sys.path:
    sys.path.insert(0, _REPO)

B, S, E = 2, 2048, 1024
HEADS, D = 16, 64
GROUPS = 4            # head groups (one per core within a batch)
HG = HEADS // GROUPS  # 4 heads per group
FG = HG * D           # 256 embed columns per group
SCALE = D ** -0.5     # 0.125

PF = 128              # partition tile
QC = 512              # free-dim chunk per matmul
NE = E // PF          # 8 contraction chunks over embed
NK = S // PF          # 16 k tiles / token tiles
NQ = S // QC          # 4 q chunks
NF = E // PF          # 8 output-feature tiles

# exp offload: heads (t, par) whose exp runs on DVE via Schraudolph bf16.
# () = all exp on ACT.
OFFLOAD = ()
# Schraudolph bf16 exp: i16 = round(s * SCH_A + SCH_B), bits are bf16.
# exp(s*SCALE) = 2^(s*SCALE*log2(e)):  A = SCALE*log2(e)*2^7,
# B = 127*2^7 - C with C tuned to minimize max rel err (C ~ 4.75 for
# round-to-nearest; equals the classic Schraudolph shift at bf16 scale).
SCH_A = SCALE * 1.4426950408889634 * 128.0
SCH_B = 127.0 * 128.0 - 4.75

_NC_CACHE = None


def _build_nc():
    """Build (once) the single-core Bass/Tile program run SPMD on all 8 cores."""
    global _NC_CACHE
    if _NC_CACHE is not None:
        return _NC_CACHE

    import concourse.bass as bass
    import concourse.tile as tile
    from concourse import bacc, mybir
    from concourse.masks import make_identity

    f32 = mybir.dt.float32
    bf16 = mybir.dt.bfloat16
    i16 = mybir.dt.int16
    Exp = mybir.ActivationFunctionType.Exp
    Mult = mybir.AluOpType.mult
    Add = mybir.AluOpType.add
    ts = bass.ts

    nc = bacc.Bacc("TRN2", target_bir_lowering=False, debug=False)

    xT_d = nc.declare_dram_parameter("xT", [PF, NE, S], bf16, isOutput=False)
    wqT_d = nc.declare_dram_parameter("wqT", [PF, NE, FG], bf16, isOutput=False)
    wkT_d = nc.declare_dram_parameter("wkT", [PF, NE, FG], bf16, isOutput=False)
    wvT_d = nc.declare_dram_parameter("wvT", [PF, NE, FG], bf16, isOutput=False)
    woT_d = nc.declare_dram_parameter("woT", [PF, 2, E], bf16, isOutput=False)
    bq_d = nc.declare_dram_parameter("bq2", [PF, 2], f32, isOutput=False)
    bk_d = nc.declare_dram_parameter("bk2", [PF, 2], f32, isOutput=False)
    bv_d = nc.declare_dram_parameter("bv1", [1, FG], bf16, isOutput=False)
    outT_d = nc.declare_dram_parameter("outT", [PF, NF, S], bf16, isOutput=True)

    with tile.TileContext(nc) as tc:
        with (
            tc.tile_pool(name="w", bufs=1) as pw,
            tc.tile_pool(name="qk", bufs=1) as pqk,
            tc.tile_pool(name="vp", bufs=1) as pv,
            tc.tile_pool(name="ctx", bufs=1) as pctx,
            tc.tile_pool(name="et", bufs=30) as pe,
            tc.tile_pool(name="nrm", bufs=4) as pn,
            tc.tile_pool(name="ow", bufs=8) as po_sb,
            tc.tile_pool(name="pss", bufs=2, space="PSUM") as pss,
            tc.tile_pool(name="pso", bufs=1, space="PSUM") as pso,
            tc.tile_pool(name="pm", bufs=1, space="PSUM") as pmisc,
        ):
            # ---- resident tensors ---------------------------------------
            x_all = pw.tile([PF, NE, S], bf16, tag="x")
            wq_all = pw.tile([PF, NE, FG], bf16, tag="wq")
            wk_all = pw.tile([PF, NE, FG], bf16, tag="wk")
            wv_all = pw.tile([PF, NE, FG], bf16, tag="wv")
            wo_all = pw.tile([PF, 2, E], bf16, tag="wo")
            bq_sb = pw.tile([PF, 2], f32, tag="bq")
            bk_sb = pw.tile([PF, 2], f32, tag="bk")
            bv_sb = pw.tile([1, FG], bf16, tag="bv")
            ones_sb = pw.tile([1, PF], bf16, tag="ones")
            ident = pw.tile([PF, PF], bf16, tag="ident")
            warm = pw.tile([PF, 1], f32, tag="warm")

            qt_sb = [pqk.tile([PF, S], bf16, tag=f"qt{t}", name=f"qt{t}") for t in range(2)]
            kt_sb = [pqk.tile([PF, S], bf16, tag=f"kt{t}", name=f"kt{t}") for t in range(2)]
            # V token-tiles: [128 tokens, head, 64+ones]
            v_all = pv.tile([PF, NK, HG, D + 1], bf16, tag="v")
            ctx_all = pctx.tile([PF, NK, HG, D], bf16, tag="ctx")
            ctxT_sb = [pctx.tile([PF, S], bf16, tag=f"ctxT{j}", name=f"ctxT{j}") for j in range(2)]

            # ---- DMAs (few, large; ordered by first use) ----------------
            # ACT exp-table preload off the critical path
            nc.vector.memset(warm[:], 0.0)
            nc.scalar.activation(warm[:], warm[:], Exp)
            nc.sync.dma_start(x_all[:, :, 0:QC], xT_d[:, :, 0:QC])
            nc.scalar.dma_start(wq_all[:], wqT_d[:])
            nc.gpsimd.dma_start(wk_all[:], wkT_d[:])
            nc.sync.dma_start(bq_sb[:], bq_d[:])
            nc.sync.dma_start(bk_sb[:], bk_d[:])
            nc.sync.dma_start(bv_sb[:], bv_d[:])
            nc.sync.dma_start(x_all[:, :, QC:S], xT_d[:, :, QC:S])
            nc.gpsimd.dma_start(wv_all[:], wvT_d[:])
            nc.gpsimd.dma_start(wo_all[:], woT_d[:])
            nc.gpsimd.memset(ones_sb[:], 1.0)
            nc.gpsimd.memset(v_all[:, :, :, D:D + 1], 1.0)
            make_identity(nc, ident[:])

            # ---- PE work generators -------------------------------------
            def proj_qk(which, t, c):
                w_sb = wq_all if which == "q" else wk_all
                b_sb = bq_sb if which == "q" else bk_sb
                o_sb = qt_sb[t] if which == "q" else kt_sb[t]
                ps = pmisc.tile([PF, QC], f32, tag="m", name="psm")
                for e in range(NE):
                    nc.tensor.matmul(
                        ps[:],
                        w_sb[:, e, ts(t, PF)],
                        x_all[:, e, ts(c, QC)],
                        start=(e == 0),
                        stop=(e == NE - 1),
                    )
                nc.vector.tensor_scalar_add(
                    o_sb[:, ts(c, QC)], ps[:], b_sb[:, t:t + 1]
                )

            def proj_v(st):
                ps = pmisc.tile([PF, FG], f32, tag="m", name="psv")
                # bias via K=1 matmul: ones^T @ bv broadcasts bv over tokens
                nc.tensor.matmul(
                    ps[:], ones_sb[:, 0:PF], bv_sb[:], start=True, stop=False
                )
                for e in range(NE):
                    nc.tensor.matmul(
                        ps[:],
                        x_all[:, e, ts(st, PF)],
                        wv_all[:, e, :],
                        start=False,
                        stop=(e == NE - 1),
                    )
                nc.vector.tensor_copy(v_all[:, st, :, 0:D], ps[:])

            # exp tile bookkeeping: et_map[(half, j, t, par, kt)] = (tile, bank)
            et_map = {}

            def attnv_chain(half, j, t, par, sub):
                h = 2 * t + par
                qs = j * 4 + sub
                qt = half * 8 + qs
                po = pso.tile([PF, D + 1], f32, tag="po", name="po")
                for kt in range(NK):
                    e_t, bank = et_map[(half, j, t, par, kt)]
                    nc.tensor.matmul(
                        po[:],
                        e_t[:, bank, ts(sub, PF)],
                        v_all[:, kt, h, :],
                        start=(kt == 0),
                        stop=(kt == NK - 1),
                    )
                r = pn.tile([PF, 1], f32, tag="r", name="r")
                nc.vector.reciprocal(r[:], po[:, D:D + 1])
                nc.vector.tensor_scalar_mul(ctx_all[:, qt, h, :], po[:, 0:D], r[:])

            def transpose_qt(qt):
                for j2 in range(2):
                    ptr = pmisc.tile([PF, PF], bf16, tag="m", name="ptr")
                    nc.tensor.transpose(
                        ptr[:], ctx_all[:, qt, 2 * j2:2 * j2 + 2, :], ident[:]
                    )
                    nc.vector.tensor_copy(ctxT_sb[j2][:, ts(qt, PF)], ptr[:])

            odma = [0]

            def outproj(c, ft):
                ps = pmisc.tile([PF, QC], f32, tag="m", name="pso2")
                for e in range(2):
                    nc.tensor.matmul(
                        ps[:],
                        wo_all[:, e, ts(ft, PF)],
                        ctxT_sb[e][:, ts(c, QC)],
                        start=(e == 0),
                        stop=(e == 1),
                    )
                ot = po_sb.tile([PF, QC], bf16, tag="ot", name="ot")
                nc.vector.tensor_copy(ot[:], ps[:])
                odma[0] += 1
                eng = (nc.sync, nc.gpsimd)[odma[0] % 2]
                eng.dma_start(outT_d[:, ft, ts(c, QC)], ot[:])

            # ---- filler scheduler ---------------------------------------
            # named idempotent work items; a deque gives default priority
            # order, force(key) emits a specific item immediately (for data
            # requirements of the score units).
            import collections
            work = {}
            fillers = collections.deque()
            done_keys = set()

            def add_work(key, cost, fn, queue=True):
                work[key] = (cost, fn)
                if queue:
                    fillers.append(key)

            def emit(key):
                if key in done_keys:
                    return 0
                cost, fn = work[key]
                fn()
                done_keys.add(key)
                _after_emit(key)
                return cost

            force = emit

            def pump(budget):
                while budget > 0 and fillers:
                    budget -= emit(fillers.popleft())
                return budget

            # transpose readiness: qt -> remaining attnV chains
            qt_remaining = {qt: 4 for qt in range(NK)}
            outproj_remaining = {c: 4 for c in range(NQ)}

            def _after_emit(key):
                if key[0] == "av":
                    _, half, j, t, par, sub = key
                    qt = half * 8 + j * 4 + sub
                    qt_remaining[qt] -= 1
                    if qt_remaining[qt] == 0:
                        add_work(("tr", qt), 250, lambda qt=qt: transpose_qt(qt))
                elif key[0] == "tr":
                    qt = key[1]
                    c = qt // 4
                    outproj_remaining[c] -= 1
                    if outproj_remaining[c] == 0:
                        for ft in range(NF):
                            add_work(
                                ("op", c, ft), 520, lambda c=c, ft=ft: outproj(c, ft)
                            )

            # initial projection work, ordered by first need; late Q chunks
            # are not queued (forced on demand) so attnV keeps priority.
            for which, t, c in [
                ("k", 0, 0), ("q", 0, 0), ("k", 0, 1), ("k", 1, 0), ("q", 1, 0),
                ("k", 0, 2), ("k", 1, 1), ("k", 0, 3), ("k", 1, 2), ("k", 1, 3),
            ]:
                add_work(
                    (which, t, c), 1800,
                    lambda which=which, t=t, c=c: proj_qk(which, t, c),
                )
            for st in range(NK):
                add_work(("v", st), 1100, lambda st=st: proj_v(st))
            for t, c in [(0, 1), (1, 1)]:
                add_work(("q", t, c), 1800, lambda t=t, c=c: proj_qk("q", t, c))
            for t, c in [(0, 2), (1, 2), (0, 3), (1, 3)]:
                add_work(
                    ("q", t, c), 1800, lambda t=t, c=c: proj_qk("q", t, c),
                    queue=False,
                )

            # ---- the global score-bank stream ---------------------------
            # blocks in order; banks kt-major (par-interleaved for PE row
            # pairing); chunked into 3-bank units, pss double-buffered.
            blocks = [(half, j, t) for half in (0, 1) for j in (0, 1) for t in (0, 1)]
            bank_stream = []
            for (half, j, t) in blocks:
                for kt in range(NK):
                    for par in (0, 1):
                        bank_stream.append((half, j, t, par, kt))

            UB = 3  # banks per unit
            units = [bank_stream[i:i + UB] for i in range(0, len(bank_stream), UB)]

            block_last_unit = {}
            for ui, unit in enumerate(units):
                for (half, j, t, par, kt) in unit:
                    block_last_unit[(half, j, t)] = ui

            for ui, unit in enumerate(units):
                n = len(unit)
                # data requirements (emission order)
                for (half, j, t, par, kt) in unit:
                    force(("k", t, kt // 4))
                    force(("q", t, 2 * half + j))
                ps = pss.tile([PF, UB, QC], f32, tag="pss", name="pss")
                e_t = pe.tile([PF, UB, QC], bf16, tag="et", name="et")
                for b, (half, j, t, par, kt) in enumerate(unit):
                    nc.tensor.matmul(
                        ps[:, b, :],
                        kt_sb[t][64 * par:64 * par + 64, ts(kt, PF)],
                        qt_sb[t][64 * par:64 * par + 64, ts(2 * half + j, QC)],
                        start=True,
                        stop=True,
                    )
                    et_map[(half, j, t, par, kt)] = (e_t, b)
                # exp per same-reader run
                b0 = 0
                while b0 < n:
                    off = unit[b0][2:4] in OFFLOAD  # (t, par)
                    b1 = b0 + 1
                    while b1 < n and (unit[b1][2:4] in OFFLOAD) == off:
                        b1 += 1
                    if off:
                        nc.vector.tensor_scalar(
                            e_t[:, b0:b1, :].bitcast(i16),
                            ps[:, b0:b1, :],
                            SCH_A,
                            SCH_B,
                            op0=Mult,
                            op1=Add,
                        )
                    else:
                        nc.scalar.activation(
                            e_t[:, b0:b1, :], ps[:, b0:b1, :], Exp, scale=SCALE
                        )
                    b0 = b1
                # block completion -> enqueue its attnV chains
                for blk, last in list(block_last_unit.items()):
                    if last == ui:
                        half, j, t = blk
                        for par in (0, 1):
                            for sub in range(4):
                                add_work(
                                    ("av", half, j, t, par, sub),
                                    600,
                                    lambda half=half, j=j, t=t, par=par, sub=sub: attnv_chain(
                                        half, j, t, par, sub
                                    ),
                                )
                        del block_last_unit[blk]
                pump(1250)

            # drain remaining work
            while fillers:
                emit(fillers.popleft())

    nc.compile()
    _NC_CACHE = nc
    return nc


def _fold(a, np_dtype):
    """[NE*PF, F] -> [PF, NE, F] partition-folded."""
    ne = a.shape[0] // PF
    return np.ascontiguousarray(
        a.reshape(ne, PF, a.shape[1]).transpose(1, 0, 2), dtype=np_dtype
    )


def _in_maps(x, Wq, bq, Wk, bk, Wv, bv, Wo, bo):
    """Per-core input dicts: core c = b*4 + g."""
    import ml_dtypes

    f = np.float32
    b16 = ml_dtypes.bfloat16
    maps = []
    for b in range(B):
        xT = _fold(np.ascontiguousarray(x[b].T), b16)
        for g in range(GROUPS):
            gs = g * FG
            sl = slice(gs, gs + FG)
            maps.append(
                {
                    "xT": xT,
                    "wqT": _fold(np.ascontiguousarray(Wq[sl, :].T), b16),
                    "wkT": _fold(np.ascontiguousarray(Wk[sl, :].T), b16),
                    "wvT": _fold(np.ascontiguousarray(Wv[sl, :].T), b16),
                    "woT": _fold(np.ascontiguousarray(Wo[:, sl].T), b16),
                    "bq2": np.ascontiguousarray(bq[sl].reshape(2, PF).T, dtype=f),
                    "bk2": np.ascontiguousarray(bk[sl].reshape(2, PF).T, dtype=f),
                    "bv1": np.ascontiguousarray(bv[sl].reshape(1, FG)).astype(b16),
                }
            )
    return maps


def _assemble(results, bo):
    out = np.empty((B, S, E), dtype=np.float32)
    for b in range(B):
        acc = results[b * GROUPS]["outT"].astype(np.float32, copy=True)
        for g in range(1, GROUPS):
            acc += results[b * GROUPS + g]["outT"]
        # unfold [PF, NF, S] -> [E, S]
        full = acc.transpose(1, 0, 2).reshape(E, S)
        out[b] = full.T + bo.astype(np.float32)
    return out


def kernel(x, Wq, bq, Wk, bk, Wv, bv, Wo, bo):
    from concourse.bass_utils import run_bass_kernel_spmd

    nc = _build_nc()
    maps = _in_maps(x, Wq, bq, Wk, bk, Wv, bv, Wo, bo)
    res = run_bass_kernel_spmd(nc, maps, core_ids=list(range(8)))
    return _assemble(res.results, np.asarray(bo))


# revision 20
# speedup vs baseline: 1.0758x; 1.0758x over previous
"""Multi-head attention (B=2, S=2048, E=1024, H=16, D=64) on 8 TRN2 cores.

Sharding: core c = b*4 + g  ->  batch b in {0,1}, head-group g in {0..3}
(4 heads = 256 embed columns per group).  Each core computes its group's
Q/K/V projections, attention, and the partial output projection
(outT [1024, 2048], the Wo[:, group]-contracted context).  Host sums the
4 group partials per batch, transposes, and adds bo.

Key structure (v2):
- All DRAM params partition-folded host-side so each loads in one big DMA.
- Q^T/K^T stored head-dim-major [128, 2048] per head-pair t; the scores
  matmul contracts K=64 per head using PE row tiling (tile_position rows
  0-63 / 64-127), so the two heads of a pair run CONCURRENTLY in the
  128x128 array -- no zero-padding waste.
- Score PSUM is a global stream of [128, 512] banks (one per (kt, par)),
  chunked into 3-bank tiles double-buffered in 6 PSUM banks.  exp runs as
  one ACT call per same-reader bank run (bigger calls amortize the
  352-cycle ACT overhead).
- Optionally some heads' exp is offloaded to the vector engine as a
  Schraudolph bf16 exp (single tensor_scalar: i16 = round(s*A + B),
  bit-interpreted as bf16), relieving the ACT throughput floor.
- attn.V: lhsT = exp^T chunk [128k, 128q], rhs = [V|1] [128k, 65]; PSUM
  col 64 = softmax denominator; normalize = reciprocal + tensor_scalar_mul.
- context is PE-transposed in 128x128 blocks for the output projection.
- A software scheduler interleaves projection / attn.V / transpose /
  out-proj work between score units to keep PE busy at the ACT call pace.
"""

import sys

import numpy as np

_REPO = "/opt/trn_rl_repo"
if _REPO not in sys.path:
    sys.path.insert(0, _REPO)

B, S, E = 2, 2048, 1024
HEADS, D = 16, 64
GROUPS = 4            # head groups (one per core within a batch)
HG = HEADS // GROUPS  # 4 heads per group
FG = HG * D           # 256 embed columns per group
SCALE = D ** -0.5     # 0.125

PF = 128              # partition tile
QC = 512              # free-dim chunk per matmul
NE = E // PF          # 8 contraction chunks over embed
NK = S // PF          # 16 k tiles / token tiles
NQ = S // QC          # 4 q chunks
NF = E // PF          # 8 output-feature tiles

# exp offload: heads (t, par) whose exp runs on DVE via Schraudolph bf16.
# () = all exp on ACT.
OFFLOAD = ((1, 1),)
# Schraudolph bf16 exp: i16 = round(s * SCH_A + SCH_B), bits are bf16.
# exp(s*SCALE) = 2^(s*SCALE*log2(e)):  A = SCALE*log2(e)*2^7,
# B = 127*2^7 - C with C = 5.5 calibrated on the score distribution
# (max mult err ~3.3%; end-to-end adds ~4e-3 rel err for 1 of 4 heads).
SCH_A = SCALE * 1.4426950408889634 * 128.0
SCH_B = 127.0 * 128.0 - 5.5

_NC_CACHE = None


def _build_nc():
    """Build (once) the single-core Bass/Tile program run SPMD on all 8 cores."""
    global _NC_CACHE
    if _NC_CACHE is not None:
        return _NC_CACHE

    import concourse.bass as bass
    import concourse.tile as tile
    from concourse import bacc, mybir
    from concourse.masks import make_identity

    f32 = mybir.dt.float32
    bf16 = mybir.dt.bfloat16
    i16 = mybir.dt.int16
    Exp = mybir.ActivationFunctionType.Exp
    Mult = mybir.AluOpType.mult
    Add = mybir.AluOpType.add
    ts = bass.ts

    nc = bacc.Bacc("TRN2", target_bir_lowering=False, debug=False)

    xT_d = nc.declare_dram_parameter("xT", [PF, NQ, NE, QC], bf16, isOutput=False)
    wqT_d = nc.declare_dram_parameter("wqT", [PF, NE, FG], bf16, isOutput=False)
    wkT_d = nc.declare_dram_parameter("wkT", [PF, NE, FG], bf16, isOutput=False)
    wvT_d = nc.declare_dram_parameter("wvT", [PF, NE, FG], bf16, isOutput=False)
    woT_d = nc.declare_dram_parameter("woT", [PF, 2, E], bf16, isOutput=False)
    bq_d = nc.declare_dram_parameter("bq2", [PF, 2], f32, isOutput=False)
    bk_d = nc.declare_dram_parameter("bk2", [PF, 2], f32, isOutput=False)
    bv_d = nc.declare_dram_parameter("bv1", [1, FG], bf16, isOutput=False)
    outT_d = nc.declare_dram_parameter("outT", [PF, NF, S], bf16, isOutput=True)

    with tile.TileContext(nc) as tc:
        with (
            tc.tile_pool(name="w", bufs=1) as pw,
            tc.tile_pool(name="qk", bufs=1) as pqk,
            tc.tile_pool(name="vp", bufs=1) as pv,
            tc.tile_pool(name="ctx", bufs=1) as pctx,
            tc.tile_pool(name="et", bufs=30) as pe,
            tc.tile_pool(name="nrm", bufs=4) as pn,
            tc.tile_pool(name="ow", bufs=8) as po_sb,
            tc.tile_pool(name="pss", bufs=2, space="PSUM") as pss,
            tc.tile_pool(name="pm", bufs=2, space="PSUM") as pmisc,
        ):
            # ---- resident tensors ---------------------------------------
            x_all = pw.tile([PF, NQ, NE, QC], bf16, tag="x")
            wq_all = pw.tile([PF, NE, FG], bf16, tag="wq")
            wk_all = pw.tile([PF, NE, FG], bf16, tag="wk")
            wv_all = pw.tile([PF, NE, FG], bf16, tag="wv")
            wo_all = pw.tile([PF, 2, E], bf16, tag="wo")
            bq_sb = pw.tile([PF, 2], f32, tag="bq")
            bk_sb = pw.tile([PF, 2], f32, tag="bk")
            bv_sb = pw.tile([1, FG], bf16, tag="bv")
            ones_sb = pw.tile([1, PF], bf16, tag="ones")
            ident = pw.tile([PF, PF], bf16, tag="ident")
            warm = pw.tile([PF, 1], f32, tag="warm")

            qt_sb = [pqk.tile([PF, S], bf16, tag=f"qt{t}", name=f"qt{t}") for t in range(2)]
            kt_sb = [pqk.tile([PF, S], bf16, tag=f"kt{t}", name=f"kt{t}") for t in range(2)]
            # V token-tiles: [128 tokens, head, 64+ones]
            v_all = pv.tile([PF, NK, HG, D + 1], bf16, tag="v")
            ctx_all = pctx.tile([PF, NK, HG, D], bf16, tag="ctx")
            ctxT_sb = [pctx.tile([PF, S], bf16, tag=f"ctxT{j}", name=f"ctxT{j}") for j in range(2)]

            # ---- DMAs (few, large; ordered by first use) ----------------
            # ACT exp-table preload off the critical path
            nc.vector.memset(warm[:], 0.0)
            nc.scalar.activation(warm[:], warm[:], Exp)
            nc.sync.dma_start(x_all[:, 0], xT_d[:, 0])
            nc.scalar.dma_start(wq_all[:], wqT_d[:])
            nc.gpsimd.dma_start(wk_all[:], wkT_d[:])
            nc.sync.dma_start(bq_sb[:], bq_d[:])
            nc.sync.dma_start(bk_sb[:], bk_d[:])
            nc.sync.dma_start(bv_sb[:], bv_d[:])
            nc.sync.dma_start(x_all[:, 1:NQ], xT_d[:, 1:NQ])
            nc.gpsimd.dma_start(wv_all[:], wvT_d[:])
            nc.gpsimd.dma_start(wo_all[:], woT_d[:])
            nc.gpsimd.memset(ones_sb[:], 1.0)
            nc.gpsimd.memset(v_all[:, :, :, D:D + 1], 1.0)
            make_identity(nc, ident[:])

            # ---- PE work generators -------------------------------------
            def proj_qk(which, t, c):
                w_sb = wq_all if which == "q" else wk_all
                b_sb = bq_sb if which == "q" else bk_sb
                o_sb = qt_sb[t] if which == "q" else kt_sb[t]
                ps = pmisc.tile([PF, QC], f32, tag="m", name="psm")
                for e in range(NE):
                    nc.tensor.matmul(
                        ps[:],
                        w_sb[:, e, ts(t, PF)],
                        x_all[:, c, e, :],
                        start=(e == 0),
                        stop=(e == NE - 1),
                    )
                nc.vector.tensor_scalar_add(
                    o_sb[:, ts(c, QC)], ps[:], b_sb[:, t:t + 1]
                )

            def proj_v(st):
                ps = pmisc.tile([PF, FG], f32, tag="m", name="psv")
                # bias via K=1 matmul: ones^T @ bv broadcasts bv over tokens
                nc.tensor.matmul(
                    ps[:], ones_sb[:, 0:PF], bv_sb[:], start=True, stop=False
                )
                for e in range(NE):
                    nc.tensor.matmul(
                        ps[:],
                        x_all[:, st // 4, e, ts(st % 4, PF)],
                        wv_all[:, e, :],
                        start=False,
                        stop=(e == NE - 1),
                    )
                nc.vector.tensor_copy(v_all[:, st, :, 0:D], ps[:])

            # exp tile bookkeeping: et_map[(half, j, t, par, kt)] = (tile, bank)
            et_map = {}

            def attnv_chain(half, j, t, par, sub):
                h = 2 * t + par
                qs = j * 4 + sub
                qt = half * 8 + qs
                po = pmisc.tile([PF, D + 1], f32, tag="m", name="po")
                for kt in range(NK):
                    e_t, bank = et_map[(half, j, t, par, kt)]
                    nc.tensor.matmul(
                        po[:],
                        e_t[:, bank, ts(sub, PF)],
                        v_all[:, kt, h, :],
                        start=(kt == 0),
                        stop=(kt == NK - 1),
                    )
                r = pn.tile([PF, 1], f32, tag="r", name="r")
                nc.vector.reciprocal(r[:], po[:, D:D + 1])
                nc.vector.tensor_scalar_mul(ctx_all[:, qt, h, :], po[:, 0:D], r[:])

            def transpose_qt(qt):
                for j2 in range(2):
                    ptr = pmisc.tile([PF, PF], bf16, tag="m", name="ptr")
                    nc.tensor.transpose(
                        ptr[:], ctx_all[:, qt, 2 * j2:2 * j2 + 2, :], ident[:]
                    )
                    nc.vector.tensor_copy(ctxT_sb[j2][:, ts(qt, PF)], ptr[:])

            odma = [0]

            def outproj(c, ft):
                ps = pmisc.tile([PF, QC], f32, tag="m", name="pso2")
                for e in range(2):
                    nc.tensor.matmul(
                        ps[:],
                        wo_all[:, e, ts(ft, PF)],
                        ctxT_sb[e][:, ts(c, QC)],
                        start=(e == 0),
                        stop=(e == 1),
                    )
                ot = po_sb.tile([PF, QC], bf16, tag="ot", name="ot")
                nc.vector.tensor_copy(ot[:], ps[:])
                odma[0] += 1
                eng = (nc.sync, nc.gpsimd)[odma[0] % 2]
                eng.dma_start(outT_d[:, ft, ts(c, QC)], ot[:])

            # ---- filler scheduler ---------------------------------------
            # named idempotent work items; a deque gives default priority
            # order, force(key) emits a specific item immediately (for data
            # requirements of the score units).
            import collections
            work = {}
            fillers = collections.deque()
            done_keys = set()

            def add_work(key, cost, fn, queue=True):
                work[key] = (cost, fn)
                if queue:
                    fillers.append(key)

            def emit(key):
                if key in done_keys:
                    return 0
                cost, fn = work[key]
                fn()
                done_keys.add(key)
                _after_emit(key)
                return cost

            force = emit

            def pump(budget):
                while budget > 0 and fillers:
                    budget -= emit(fillers.popleft())
                return budget

            # transpose readiness: qt -> remaining attnV chains
            qt_remaining = {qt: 4 for qt in range(NK)}
            outproj_remaining = {c: 4 for c in range(NQ)}

            def _after_emit(key):
                if key[0] == "av":
                    _, half, j, t, par, sub = key
                    qt = half * 8 + j * 4 + sub
                    qt_remaining[qt] -= 1
                    if qt_remaining[qt] == 0:
                        add_work(("tr", qt), 250, lambda qt=qt: transpose_qt(qt))
                elif key[0] == "tr":
                    qt = key[1]
                    c = qt // 4
                    outproj_remaining[c] -= 1
                    if outproj_remaining[c] == 0:
                        for ft in range(NF):
                            add_work(
                                ("op", c, ft), 520, lambda c=c, ft=ft: outproj(c, ft)
                            )

            # initial projection work, ordered by first need; late Q chunks
            # are not queued (forced on demand) so attnV keeps priority.
            for which, t, c in [
                ("k", 0, 0), ("q", 0, 0), ("k", 0, 1), ("k", 1, 0), ("q", 1, 0),
                ("k", 0, 2), ("k", 1, 1), ("k", 0, 3), ("k", 1, 2), ("k", 1, 3),
            ]:
                add_work(
                    (which, t, c), 1800,
                    lambda which=which, t=t, c=c: proj_qk(which, t, c),
                )
            for st in range(NK):
                add_work(("v", st), 1100, lambda st=st: proj_v(st))
            for t, c in [(0, 1), (1, 1)]:
                add_work(("q", t, c), 1800, lambda t=t, c=c: proj_qk("q", t, c))
            for t, c in [(0, 2), (1, 2), (0, 3), (1, 3)]:
                add_work(
                    ("q", t, c), 1800, lambda t=t, c=c: proj_qk("q", t, c),
                    queue=False,
                )

            # ---- the global score-bank stream ---------------------------
            # blocks in order; banks kt-major (par-interleaved for PE row
            # pairing); chunked into 3-bank units, pss double-buffered.
            # pair-major groups of 4: [2kp|p0, 2kp+1|p0, 2kp|p1, 2kp+1|p1]
            # -- consecutive different-row-group MMs still overlap pairwise,
            # and same-reader banks cluster for bigger exp calls.
            blocks = [(half, j, t) for half in (0, 1) for j in (0, 1) for t in (0, 1)]
            bank_stream = []
            for (half, j, t) in blocks:
                for kp in range(NK // 2):
                    for par in (0, 1):
                        for kt in (2 * kp, 2 * kp + 1):
                            bank_stream.append((half, j, t, par, kt))

            UB = 3  # banks per unit
            units = [bank_stream[i:i + UB] for i in range(0, len(bank_stream), UB)]

            block_last_unit = {}
            for ui, unit in enumerate(units):
                for (half, j, t, par, kt) in unit:
                    block_last_unit[(half, j, t)] = ui

            for ui, unit in enumerate(units):
                n = len(unit)
                # data requirements (emission order)
                for (half, j, t, par, kt) in unit:
                    force(("k", t, kt // 4))
                    force(("q", t, 2 * half + j))
                ps = pss.tile([PF, UB, QC], f32, tag="pss", name="pss")
                e_t = pe.tile([PF, UB, QC], bf16, tag="et", name="et")
                for b, (half, j, t, par, kt) in enumerate(unit):
                    nc.tensor.matmul(
                        ps[:, b, :],
                        kt_sb[t][64 * par:64 * par + 64, ts(kt, PF)],
                        qt_sb[t][64 * par:64 * par + 64, ts(2 * half + j, QC)],
                        start=True,
                        stop=True,
                    )
                    et_map[(half, j, t, par, kt)] = (e_t, b)
                # exp per same-reader run
                b0 = 0
                while b0 < n:
                    off = unit[b0][2:4] in OFFLOAD  # (t, par)
                    b1 = b0 + 1
                    while b1 < n and (unit[b1][2:4] in OFFLOAD) == off:
                        b1 += 1
                    if off:
                        nc.vector.tensor_scalar(
                            e_t[:, b0:b1, :].bitcast(i16),
                            ps[:, b0:b1, :],
                            SCH_A,
                            SCH_B,
                            op0=Mult,
                            op1=Add,
                        )
                    else:
                        nc.scalar.activation(
                            e_t[:, b0:b1, :], ps[:, b0:b1, :], Exp, scale=SCALE
                        )
                    b0 = b1
                # block completion -> enqueue its attnV chains
                for blk, last in list(block_last_unit.items()):
                    if last == ui:
                        half, j, t = blk
                        for par in (0, 1):
                            for sub in range(4):
                                add_work(
                                    ("av", half, j, t, par, sub),
                                    600,
                                    lambda half=half, j=j, t=t, par=par, sub=sub: attnv_chain(
                                        half, j, t, par, sub
                                    ),
                                )
                        del block_last_unit[blk]
                # late units pump harder so the PE-only tail stays short
                pump(1250 if ui < 56 else (2000 if ui < 71 else 3200))

            # drain remaining work
            while fillers:
                emit(fillers.popleft())

    nc.compile()
    _NC_CACHE = nc
    return nc


def _fold(a, np_dtype):
    """[NE*PF, F] -> [PF, NE, F] partition-folded."""
    ne = a.shape[0] // PF
    return np.ascontiguousarray(
        a.reshape(ne, PF, a.shape[1]).transpose(1, 0, 2), dtype=np_dtype
    )


def _in_maps(x, Wq, bq, Wk, bk, Wv, bv, Wo, bo):
    """Per-core input dicts: core c = b*4 + g."""
    import ml_dtypes

    f = np.float32
    b16 = ml_dtypes.bfloat16
    maps = []
    for b in range(B):
        # x^T [E, S] -> [PF, NQ, NE, QC]: element (128e+p, 512c+s) -> (p,c,e,s)
        xT = np.ascontiguousarray(
            x[b].T.reshape(NE, PF, NQ, QC).transpose(1, 2, 0, 3)
        ).astype(b16)
        for g in range(GROUPS):
            gs = g * FG
            sl = slice(gs, gs + FG)
            maps.append(
                {
                    "xT": xT,
                    "wqT": _fold(np.ascontiguousarray(Wq[sl, :].T), b16),
                    "wkT": _fold(np.ascontiguousarray(Wk[sl, :].T), b16),
                    "wvT": _fold(np.ascontiguousarray(Wv[sl, :].T), b16),
                    "woT": _fold(np.ascontiguousarray(Wo[:, sl].T), b16),
                    "bq2": np.ascontiguousarray(bq[sl].reshape(2, PF).T, dtype=f),
                    "bk2": np.ascontiguousarray(bk[sl].reshape(2, PF).T, dtype=f),
                    "bv1": np.ascontiguousarray(bv[sl].reshape(1, FG)).astype(b16),
                }
            )
    return maps


def _assemble(results, bo):
    out = np.empty((B, S, E), dtype=np.float32)
    for b in range(B):
        acc = results[b * GROUPS]["outT"].astype(np.float32, copy=True)
        for g in range(1, GROUPS):
            acc += results[b * GROUPS + g]["outT"]
        # unfold [PF, NF, S] -> [E, S]
        full = acc.transpose(1, 0, 2).reshape(E, S)
        out[b] = full.T + bo.astype(np.float32)
    return out


def kernel(x, Wq, bq, Wk, bk, Wv, bv, Wo, bo):
    from concourse.bass_utils import run_bass_kernel_spmd

    nc = _build_nc()
    maps = _in_maps(x, Wq, bq, Wk, bk, Wv, bv, Wo, bo)
    res = run_bass_kernel_spmd(nc, maps, core_ids=list(range(8)))
    return _assemble(res.results, np.asarray(bo))
